# revision 1
# baseline (speedup 1.0000x reference)
"""TRN2 Bass kernel for nn_CosFreqEncoding: out = ((x @ W.T) @ cos_basis) / max.

Strategy: data-parallel over batch across 8 NeuronCores. Each core computes
its 512-row shard of both GEMMs in fp32r (e8m11, full TensorE rate), a local
max, one scalar AllReduce(max), then scales and writes its output shard.

Layouts (host-prepped so no on-chip transposes are needed):
  GEMM1: xfT[f, m] += W.T[l, f].T @ x.T[l, m]   (lhsT = W.T block, rhs = x.T)
  GEMM2: out[m, l2] += xfT[f, m].T @ cos[f, l2] (lhsT = xfT slice, rhs = cos)

Self-contained: hardcodes shapes from the problem spec.
"""
import numpy as np

import concourse.bass as bass
import concourse.bacc as bacc
import concourse.mybir as mybir
import concourse.tile as tile
import concourse.bass_utils as bass_utils

N_CORES = 8
B, L, F = 4096, 2048, 2074
FP = 2176               # F padded to 17 full 128-tiles
BS = B // N_CORES       # 512 batch rows per core
LT = L // 128           # 16 l-tiles (GEMM1 contraction)
FT = FP // 128          # 17 f-tiles
MT = BS // 128          # 4 m-tiles
CK = L // 512           # 4 output column chunks of 512
F32 = mybir.dt.float32
F32R = mybir.dt.float32r
NEG_INF = -3.0e38


def _to_fp32r(a: np.ndarray) -> np.ndarray:
    """Round fp32 to fp32r (e8m11): RNE at mantissa bit 12, low bits zeroed."""
    b = np.ascontiguousarray(a, dtype=np.float32).view(np.uint32).astype(np.uint64)
    b = b + 0x7FF + ((b >> 12) & 1)
    return (b & 0xFFFF_F000).astype(np.uint32).view(np.float32)


def _armax(nc, sp, dp, vm_slice, tag, q):
    """Local max of vm_slice -> scalar -> AllReduce(max); returns dram out."""
    g = sp.tile([1, 1], F32, name=f"g_{tag}")
    q.reduce_max(g[:], vm_slice, axis=mybir.AxisListType.XYZWC)
    cc_in = dp.tile([1], F32, name=f"ccin_{tag}")
    cc_out = dp.tile([1], F32, name=f"ccout_{tag}")
    q.dma_start(cc_in[:], g[:, 0])
    nc.gpsimd.collective_compute(
        "AllReduce", mybir.AluOpType.max,
        replica_groups=[list(range(N_CORES))],
        ins=[cc_in[:]], outs=[cc_out[:]])
    return cc_out


def _emit(nc, tc, xT, Wb, cosb, out, variant):
    with (
        tc.tile_pool(name="xp", bufs=1) as xp,
        tc.tile_pool(name="wp", bufs=4) as wp,
        tc.tile_pool(name="xfp", bufs=1) as xfp,
        tc.tile_pool(name="cp", bufs=12) as cp,
        tc.tile_pool(name="op", bufs=1) as op,
        tc.tile_pool(name="sp", bufs=1) as sp,
        tc.tile_pool(name="ps1", bufs=2, space="PSUM") as ps1,
        tc.tile_pool(name="ps2", bufs=6, space="PSUM") as ps2,
        tc.tile_pool(name="dp", bufs=1, space="DRAM") as dp,
    ):
        # DMA issuers round-robin: each engine owns its own DGE queue, and a
        # single queue's descriptor-gen (~600ns/transfer) caps at ~110GB/s.
        qs = [nc.sync, nc.scalar, nc.gpsimd]

        # resident x.T tiles: [128 l, 512 m] x 16
        xt = []
        for li in range(LT):
            t = xp.tile([128, BS], F32R, name=f"xt{li}")
            (nc.sync if li % 2 == 0 else nc.gpsimd).dma_start(t[:], xT[li])
            xt.append(t)

        if variant == "io":
            w0 = wp.tile([128, LT * 128], F32R, tag="w")
            nc.sync.dma_start(
                w0[:].rearrange("p (li b) -> p li b", li=LT), Wb[0])
            c0 = cp.tile([128, 512], F32R, tag="cos")
            nc.sync.dma_start(c0[:], cosb[0, 0])
            for li in range(4):
                nc.sync.dma_start(out[li * 128:(li + 1) * 128, 0:512],
                                  xt[li][:].bitcast(F32))
            nc.sync.dma_start(out[0:128, 512:640],
                              w0[:, 0:128].bitcast(F32))
            nc.sync.dma_start(out[0:128, 1024:1536], c0[:].bitcast(F32))
            return

        # GEMM1: xfT[f-tile] [128 f, 512 m]. W streamed as one 1MB DMA per
        # f-tile column (16 blocks) to amortize DGE descriptor-gen.
        xf = [xfp.tile([128, BS], F32R, name=f"xf{fi}") for fi in range(FT)]
        for fi in range(FT):
            ps = ps1.tile([128, BS], F32, tag="g1")
            wcol = wp.tile([128, LT * 128], F32R, tag="w")
            wv = wcol[:].rearrange("p (li b) -> p li b", li=LT)
            (nc.scalar if fi % 2 == 0 else nc.sync).dma_start(wv, Wb[fi])
            for li in range(LT):
                nc.tensor.matmul(ps[:], wcol[:, li * 128:(li + 1) * 128],
                                 xt[li][:],
                                 start=(li == 0), stop=(li == LT - 1))
            # cast+round fp32 -> fp32r while copying out of PSUM
            nc.vector.tensor_copy(xf[fi][:], ps[:])

        if variant == "g1":
            for ci in range(4):
                nc.sync.dma_start(out[0:128, ci * 512:(ci + 1) * 512],
                                  xf[ci][:].bitcast(F32))
            return

        # GEMM2 + fused local max
        ot = [op.tile([128, L], F32, name=f"ot{mi}") for mi in range(MT)]
        vmaxes = sp.tile([128, MT * CK], F32)
        for ci in range(CK):
            pst = [ps2.tile([128, 512], F32, tag="g2", name=f"ps2_{ci}_{mi}")
                   for mi in range(MT)]
            for fi in range(FT):
                c = cp.tile([128, 512], F32R, tag="cos")
                qs[(ci * FT + fi) % 2].dma_start(c[:], cosb[ci, fi])
                for mi in range(MT):
                    nc.tensor.matmul(
                        pst[mi][:], xf[fi][:, mi * 128:(mi + 1) * 128], c[:],
                        start=(fi == 0), stop=(fi == FT - 1))
            for mi in range(MT):
                idx = ci * MT + mi
                osl = ot[mi][:, ci * 512:(ci + 1) * 512]
                nc.vector.tensor_copy(osl, pst[mi][:])
                if variant != "g2a":
                    nc.vector.reduce_max(vmaxes[:, idx:idx + 1], osl,
                                         axis=mybir.AxisListType.X)
            if variant == "full" and ci == 1:
                # stage-1 AllReduce over chunks 0..1, hidden under chunks 2-3
                cc1_out = _armax(nc, sp, dp, vmaxes[:, 0:2 * MT], "s1",
                                 nc.gpsimd)

        if variant in ("nonorm", "g2a"):
            for mi in range(MT):
                nc.sync.dma_start(out[mi * 128:(mi + 1) * 128, :], ot[mi][:])
            return

        # stage-2 AllReduce over the last chunk's maxes, then combine
        cc2_out = _armax(nc, sp, dp, vmaxes[:, 2 * MT:CK * MT], "s2",
                         nc.gpsimd)
        gbc1 = sp.tile([128, 1], F32)
        nc.sync.dma_start(gbc1[:], cc1_out[:].partition_broadcast(128))
        gbc2 = sp.tile([128, 1], F32)
        nc.scalar.dma_start(gbc2[:], cc2_out[:].partition_broadcast(128))
        gbc = sp.tile([128, 1], F32)
        nc.vector.tensor_scalar_max(gbc[:], gbc1[:], gbc2[:, 0:1])
        rbc = sp.tile([128, 1], F32)
        nc.vector.reciprocal(rbc[:], gbc[:])

        # scale + store (chunked so DMA of one slice overlaps mul of the next)
        for mi in range(MT):
            for ci in range(CK):
                sl = slice(ci * 512, (ci + 1) * 512)
                nc.vector.tensor_scalar_mul(ot[mi][:, sl], ot[mi][:, sl],
                                            rbc[:, 0:1])
                qs[(mi * CK + ci) % 3].dma_start(
                    out[mi * 128:(mi + 1) * 128, sl], ot[mi][:, sl])


def _build(variant="full"):
    nc = bacc.Bacc("TRN2", target_bir_lowering=False, debug=False,
                   num_devices=N_CORES)
    xT = nc.dram_tensor("xT", [LT, 128, BS], F32R, kind="ExternalInput")
    Wb = nc.dram_tensor("Wb", [FT, 128, LT, 128], F32R, kind="ExternalInput")
    cosb = nc.dram_tensor("cosb", [CK, FT, 128, 512], F32R, kind="ExternalInput")
    out = nc.dram_tensor("out", [BS, L], F32, kind="ExternalOutput")
    with tile.TileContext(nc) as tc:
        _emit(nc, tc, xT, Wb, cosb, out, variant)
    nc.compile()
    return nc


_cached_nc = None


def _get_nc():
    global _cached_nc
    if _cached_nc is None:
        _cached_nc = _build()
    return _cached_nc


def _prep_inputs(x, W, cos_basis):
    x = np.ascontiguousarray(x, dtype=np.float32)
    W = np.ascontiguousarray(W, dtype=np.float32)
    cos = np.ascontiguousarray(cos_basis, dtype=np.float32)
    # pad freq dim to FP with zeros
    Wp = np.zeros((FP, L), dtype=np.float32)
    Wp[:F] = W
    cosp = np.zeros((FP, L), dtype=np.float32)
    cosp[:F] = cos
    # Wb[fi, p, li, b] = W.T[li*128+p, fi*128+b] = Wp[fi*128+b, li*128+p]
    Wb = np.ascontiguousarray(
        Wp.reshape(FT, 128, LT, 128).transpose(0, 3, 2, 1))
    Wb = _to_fp32r(Wb)
    # cosb[ci, fi, a, n] = cosp[fi*128+a, ci*512+n]
    cosb = np.ascontiguousarray(
        cosp.reshape(FT, 128, CK, 512).transpose(2, 0, 1, 3))
    cosb = _to_fp32r(cosb)
    xTs = []
    for i in range(N_CORES):
        sh = np.ascontiguousarray(x[i * BS:(i + 1) * BS].T)  # (L, BS)
        xTs.append(_to_fp32r(sh.reshape(LT, 128, BS)))
    return xTs, Wb, cosb


def kernel(x, W, cos_basis, _trace=False, _trace_kwargs=None):
    xTs, Wb, cosb = _prep_inputs(x, W, cos_basis)
    nc = _get_nc()
    in_maps = [{"xT": xTs[i], "Wb": Wb, "cosb": cosb} for i in range(N_CORES)]
    res = bass_utils.run_bass_kernel_spmd(
        nc, in_maps, core_ids=list(range(N_CORES)), trace=_trace,
        **(_trace_kwargs or {}))
    out = np.concatenate([res.results[i]["out"] for i in range(N_CORES)],
                         axis=0)
    if _trace:
        kernel.last_result = res
    return out



# revision 11
# speedup vs baseline: 1.2722x; 1.2722x over previous
"""TRN2 Bass kernel for nn_CosFreqEncoding via reassociation:
out = x @ (W.T @ cos_basis) / max.

Strategy: shard the OUTPUT COLUMNS across the 8 cores. Core i computes
M_i = (W.T @ cos)[:, i*256:(i+1)*256] from the full W and its cos column
slice (phase A, 1/8 of the M GEMM, no redundancy), then
outT_i = M_i.T-oriented GEMM against the full x.T (phase B), covering
out[:, i*256:(i+1)*256] for ALL 4096 batch rows. Total PE rows drop 28%
vs the two-GEMM data-parallel form (201k vs 279k) and no tensor-sized
collective is needed -- only the scalar AllReduce(max) for normalization.

Layouts (all natural, no host transposes except x.T):
  A: psumA[l, c]  += W[f, l-slice].T @ cosS[f, c]      (both f-major)
  B: psumB[c, m]  += Msb[l, c-slice].T @ xT[l, m]      (both l-major)

bf16 operands, f32 accumulate. GpSimd only triggers collectives and does
two tiny cross-lane reduces (no SWDGE DMAs -> short epilogue drain).

Self-contained: hardcodes shapes from the problem spec.
"""
import ml_dtypes
import numpy as np

import concourse.bass as bass
import concourse.bacc as bacc
import concourse.mybir as mybir
import concourse.tile as tile
import concourse.bass_utils as bass_utils

N_CORES = 8
B, L, F = 4096, 2048, 2074
FP = 2176               # F padded to 17 full 128-tiles
CS = L // N_CORES       # 256 output columns per core
LT = L // 128           # 16 l-tiles
FT = FP // 128          # 17 f-tiles (phase A contraction)
F32 = mybir.dt.float32
BF16 = mybir.dt.bfloat16


def _armax(nc, sp, dp, vm_slice, tag, dmaq):
    """Local max of vm_slice -> scalar -> AllReduce(max); returns dram out."""
    g = sp.tile([1, 1], F32, name=f"g_{tag}")
    nc.gpsimd.reduce_max(g[:], vm_slice, axis=mybir.AxisListType.XYZWC)
    cc_in = dp.tile([1], F32, name=f"ccin_{tag}")
    cc_out = dp.tile([1], F32, name=f"ccout_{tag}")
    dmaq.dma_start(cc_in[:], g[:, 0])
    nc.gpsimd.collective_compute(
        "AllReduce", mybir.AluOpType.max,
        replica_groups=[list(range(N_CORES))],
        ins=[cc_in[:]], outs=[cc_out[:]])
    return cc_out


def _emit(nc, tc, xT, Wb, cosS, out):
    with (
        tc.tile_pool(name="wp", bufs=2) as wp,
        tc.tile_pool(name="cp", bufs=1) as cp,
        tc.tile_pool(name="mp", bufs=1) as mp,
        tc.tile_pool(name="xp", bufs=6) as xp,
        tc.tile_pool(name="op", bufs=1) as op,
        tc.tile_pool(name="sp", bufs=1) as sp,
        tc.tile_pool(name="ps", bufs=8, space="PSUM") as ps,
        tc.tile_pool(name="dp", bufs=1, space="DRAM") as dp,
    ):
        qs = [nc.sync, nc.scalar]

        # Warmup AllReduce: absorbs the cross-core bootstrap barrier and the
        # first-collective setup cost.
        wz = sp.tile([1, 1], F32, name="warm_z")
        nc.vector.memset(wz[:], 0.0)
        warm_in = dp.tile([1], F32, name="warm_in")
        warm_out = dp.tile([1], F32, name="warm_out")
        nc.sync.dma_start(warm_in[:], wz[:, 0])
        nc.gpsimd.collective_compute(
            "AllReduce", mybir.AluOpType.max,
            replica_groups=[list(range(N_CORES))],
            ins=[warm_in[:]], outs=[warm_out[:]])

        # ---- Phase A: M_i[2048, 256] = W.T @ cosS, two halves of 8 l-tiles.
        # cos slice resident in SBUF; W streamed per half as a few large
        # per-partition-contiguous DMAs. First-needed pieces (cos ft0 on
        # scalar, W ft0 on sync) issue ahead of the bulk so the first matmul
        # fires early. Each accumulation owns a full PSUM bank (matmul
        # start=True clears the whole bank).
        cbig = cp.tile([128, FT * CS], BF16, name="cosr")
        nc.scalar.dma_start(cbig[:, 0:CS], cosS[:, 0:CS])
        WCH = [(0, 1), (1, 3), (3, 7), (7, 12), (12, 17)]
        msb = [mp.tile([128, CS], BF16, name=f"msb{lt}") for lt in range(LT)]
        wbigs = [wp.tile([128, FT * 8 * 128], BF16, tag="w", name=f"wbig{h}")
                 for h in range(2)]
        nc.sync.dma_start(wbigs[0][:, 0:1024], Wb[0, :, 0:1024])
        nc.scalar.dma_start(wbigs[0][:, 1024:3072], Wb[0, :, 1024:3072])
        nc.sync.dma_start(cbig[:, CS:4 * CS], cosS[:, CS:4 * CS])
        nc.scalar.dma_start(cbig[:, 4 * CS:FT * CS], cosS[:, 4 * CS:FT * CS])
        for h in range(2):
            pa = [ps.tile([128, 512], F32, tag="ps", name=f"pa{h}_{k}")
                  for k in range(8)]
            wbig = wbigs[h]
            for n, (f0, f1) in enumerate(WCH):
                if h == 0 and n < 2:
                    continue  # issued above, ahead of the bulk cos
                qs[n % 2].dma_start(wbig[:, f0 * 1024:f1 * 1024],
                                    Wb[h, :, f0 * 1024:f1 * 1024])
            for ft in range(FT):
                for lk in range(8):
                    nc.tensor.matmul(
                        pa[lk][:, 0:CS],
                        wbig[:, ft * 1024 + lk * 128:ft * 1024 + (lk + 1) * 128],
                        cbig[:, ft * CS:(ft + 1) * CS],
                        start=(ft == 0), stop=(ft == FT - 1))
            for k in range(8):
                if k % 2 == 0:
                    nc.vector.tensor_copy(msb[h * 8 + k][:], pa[k][:, 0:CS])
                else:
                    nc.scalar.copy(msb[h * 8 + k][:], pa[k][:, 0:CS])

        # ---- Phase B: outT[256, 4096] = M_i.T @ x, two passes of 4 m-chunks
        ot = [op.tile([128, B], F32, name=f"ot{ct}") for ct in range(2)]
        vmaxes = sp.tile([128, 16], F32)
        cco1 = dp.tile([1], F32, name="cco1")
        cco2 = dp.tile([1], F32, name="cco2")
        for p in range(2):
            pb = [ps.tile([128, 512], F32, tag="ps", name=f"pb{p}_{j}")
                  for j in range(8)]
            for lt in range(LT):
                xt = xp.tile([128, 2048], BF16, tag="x")
                qs[lt % 2].dma_start(xt[:], xT[lt, :, p * 2048:(p + 1) * 2048])
                for ct in range(2):
                    lhsT = msb[lt][:, ct * 128:(ct + 1) * 128]
                    for mc in range(4):
                        nc.tensor.matmul(
                            pb[ct * 4 + mc][:], lhsT,
                            xt[:, mc * 512:(mc + 1) * 512],
                            start=(lt == 0), stop=(lt == LT - 1))
            if p == 0:
                # reduce maxes straight from PSUM (vector) so the stage-1
                # AllReduce triggers early
                for j in range(8):
                    nc.vector.reduce_max(vmaxes[:, j:j + 1], pb[j][:],
                                         axis=mybir.AxisListType.X)
                g1 = sp.tile([1, 1], F32, name="g_s1")
                nc.gpsimd.reduce_max(g1[:], vmaxes[:, 0:8],
                                     axis=mybir.AxisListType.XYZWC)
                cc_in1 = dp.tile([1], F32, name="ccin1")
                nc.sync.dma_start(cc_in1[:], g1[:, 0])
                nc.gpsimd.collective_compute(
                    "AllReduce", mybir.AluOpType.max,
                    replica_groups=[list(range(N_CORES))],
                    ins=[cc_in1[:]], outs=[cco1[:]])
            else:
                for j in range(8):
                    nc.vector.reduce_max(vmaxes[:, 8 + j:9 + j], pb[j][:],
                                         axis=mybir.AxisListType.X)
                g2 = sp.tile([1, 1], F32, name="g_s2")
                nc.gpsimd.reduce_max(g2[:], vmaxes[:, 8:16],
                                     axis=mybir.AxisListType.XYZWC)
                cc_in2 = dp.tile([1], F32, name="ccin2")
                nc.scalar.dma_start(cc_in2[:], g2[:, 0])
                nc.gpsimd.collective_compute(
                    "AllReduce", mybir.AluOpType.max,
                    replica_groups=[list(range(N_CORES))],
                    ins=[cc_in2[:]], outs=[cco2[:]])
            for j in range(8):
                ct, mc = j // 4, j % 4
                k = p * 4 + mc
                osl = ot[ct][:, k * 512:(k + 1) * 512]
                if j % 2 == 0:
                    nc.vector.tensor_copy(osl, pb[j][:])
                else:
                    nc.scalar.copy(osl, pb[j][:])

        # combine both stage maxes
        gbc2 = sp.tile([128, 2], F32)
        nc.sync.dma_start(gbc2[:, 0:1], cco1[:].partition_broadcast(128))
        nc.scalar.dma_start(gbc2[:, 1:2], cco2[:].partition_broadcast(128))
        gbc = sp.tile([128, 1], F32)
        nc.vector.reduce_max(gbc[:], gbc2[:], axis=mybir.AxisListType.X)
        rbc = sp.tile([128, 1], F32)
        nc.vector.reciprocal(rbc[:], gbc[:])

        # scale (f32 -> bf16) + store in [128, 1024] chunks; muls split
        # DVE/Activation (5/3 balances their rates), stores all on sync
        ots = [op.tile([128, B], BF16, name=f"ots{ct}") for ct in range(2)]
        order = [(ct, kk) for ct in range(2) for kk in range(4)]
        for n, (ct, kk) in enumerate(order):
            sl = slice(kk * 1024, (kk + 1) * 1024)
            if n in (1, 4, 6):
                nc.scalar.mul(ots[ct][:, sl], ot[ct][:, sl], rbc[:, 0:1])
            else:
                nc.vector.tensor_scalar_mul(ots[ct][:, sl], ot[ct][:, sl],
                                            rbc[:, 0:1])
            nc.sync.dma_start(
                out[ct * 128:(ct + 1) * 128, sl], ots[ct][:, sl])


def _build():
    nc = bacc.Bacc("TRN2", target_bir_lowering=False, debug=False,
                   num_devices=N_CORES)
    xT = nc.dram_tensor("xT", [LT, 128, B], BF16, kind="ExternalInput")
    # Wb[h, p, ft*8*128 + lk*128 + b] = Wp[ft*128+p, (h*8+lk)*128+b]
    Wb = nc.dram_tensor("Wb", [2, 128, FT * 8 * 128], BF16,
                        kind="ExternalInput")
    # cosS[p, ft*CS + c] = cosp[ft*128+p, core_lo + c]
    cosS = nc.dram_tensor("cosS", [128, FT * CS], BF16, kind="ExternalInput")
    out = nc.dram_tensor("out", [CS, B], BF16, kind="ExternalOutput")
    with tile.TileContext(nc) as tc:
        _emit(nc, tc, xT, Wb, cosS, out)
    nc.compile()
    return nc


_cached_nc = None


def _get_nc():
    global _cached_nc
    if _cached_nc is None:
        _cached_nc = _build()
    return _cached_nc


def _bf16(a: np.ndarray) -> np.ndarray:
    return np.ascontiguousarray(a, dtype=np.float32).astype(ml_dtypes.bfloat16)


def _prep_inputs(x, W, cos_basis):
    x = np.ascontiguousarray(x, dtype=np.float32)
    W = np.ascontiguousarray(W, dtype=np.float32)
    cos = np.ascontiguousarray(cos_basis, dtype=np.float32)
    Wp = np.zeros((FP, L), dtype=np.float32)
    Wp[:F] = W
    cosp = np.zeros((FP, L), dtype=np.float32)
    cosp[:F] = cos
    # Wb[h, p, (ft, lk, b)] = Wp[ft*128+p, (h*8+lk)*128+b]
    W4 = Wp.reshape(FT, 128, LT, 128)
    Wb = _bf16(np.stack([
        np.ascontiguousarray(
            W4[:, :, h * 8:(h + 1) * 8, :].transpose(1, 0, 2, 3)
        ).reshape(128, FT * 8 * 128)
        for h in range(2)]))
    xTf = _bf16(np.ascontiguousarray(x.T).reshape(LT, 128, B))
    # cosS[p, (ft, c)] = cosp[ft*128+p, i*CS+c]
    cosSs = [_bf16(np.ascontiguousarray(
        cosp[:, i * CS:(i + 1) * CS].reshape(FT, 128, CS).transpose(1, 0, 2)
    ).reshape(128, FT * CS)) for i in range(N_CORES)]
    return xTf, Wb, cosSs


def kernel(x, W, cos_basis, _trace=False, _trace_kwargs=None):
    xTf, Wb, cosSs = _prep_inputs(x, W, cos_basis)
    nc = _get_nc()
    in_maps = [{"xT": xTf, "Wb": Wb, "cosS": cosSs[i]}
               for i in range(N_CORES)]
    res = bass_utils.run_bass_kernel_spmd(
        nc, in_maps, core_ids=list(range(N_CORES)), trace=_trace,
        **(_trace_kwargs or {}))
    full = np.empty((B, L), dtype=np.float32)
    for i in range(N_CORES):
        full[:, i * CS:(i + 1) * CS] = res.results[i]["out"].astype(np.float32).T
    if _trace:
        kernel.last_result = res
    return full


# revision 15
# speedup vs baseline: 1.4291x; 1.1233x over previous
"""TRN2 Bass kernel for nn_CosFreqEncoding via reassociation:
out = x @ (W.T @ cos_basis) / max.

Strategy: shard the OUTPUT COLUMNS across the 8 cores. Core i computes
M_i = (W.T @ cos)[:, i*256:(i+1)*256] from the full W and its cos column
slice (phase A, 1/8 of the M GEMM, no redundancy), then
outT_i = M_i.T-oriented GEMM against the full x.T (phase B), covering
out[:, i*256:(i+1)*256] for ALL 4096 batch rows. Total PE rows drop 28%
vs the two-GEMM data-parallel form (201k vs 279k) and no tensor-sized
collective is needed -- only the scalar AllReduce(max) for normalization.

Layouts (all natural, no host transposes except x.T):
  A: psumA[l, c]  += W[f, l-slice].T @ cosS[f, c]      (both f-major)
  B: psumB[c, m]  += Msb[l, c-slice].T @ xT[l, m]      (both l-major)

bf16 operands, f32 accumulate. GpSimd only triggers collectives and does
two tiny cross-lane reduces (no SWDGE DMAs -> short epilogue drain).

Self-contained: hardcodes shapes from the problem spec.
"""
import ml_dtypes
import numpy as np

import concourse.bass as bass
import concourse.bacc as bacc
import concourse.mybir as mybir
import concourse.tile as tile
import concourse.bass_utils as bass_utils

N_CORES = 8
B, L, F = 4096, 2048, 2074
FP = 2176               # F padded to 17 full 128-tiles
CS = L // N_CORES       # 256 output columns per core
LT = L // 128           # 16 l-tiles
FT = FP // 128          # 17 f-tiles (phase A contraction)
F32 = mybir.dt.float32
BF16 = mybir.dt.bfloat16


def _armax(nc, sp, dp, vm_slice, tag, dmaq):
    """Local max of vm_slice -> scalar -> AllReduce(max); returns dram out."""
    g = sp.tile([1, 1], F32, name=f"g_{tag}")
    nc.gpsimd.reduce_max(g[:], vm_slice, axis=mybir.AxisListType.XYZWC)
    cc_in = dp.tile([1], F32, name=f"ccin_{tag}")
    cc_out = dp.tile([1], F32, name=f"ccout_{tag}")
    dmaq.dma_start(cc_in[:], g[:, 0])
    nc.gpsimd.collective_compute(
        "AllReduce", mybir.AluOpType.max,
        replica_groups=[list(range(N_CORES))],
        ins=[cc_in[:]], outs=[cc_out[:]])
    return cc_out


def _emit(nc, tc, xT, Wb, cosS, out):
    with (
        tc.tile_pool(name="wp", bufs=2) as wp,
        tc.tile_pool(name="cp", bufs=1) as cp,
        tc.tile_pool(name="mp", bufs=1) as mp,
        tc.tile_pool(name="xp", bufs=6) as xp,
        tc.tile_pool(name="op", bufs=1) as op,
        tc.tile_pool(name="sp", bufs=1) as sp,
        tc.tile_pool(name="ps", bufs=8, space="PSUM") as ps,
        tc.tile_pool(name="dp", bufs=1, space="DRAM") as dp,
    ):
        qs = [nc.sync, nc.scalar]

        # Warmup AllReduce: absorbs the cross-core bootstrap barrier and the
        # first-collective setup cost.
        wz = sp.tile([1, 1], F32, name="warm_z")
        nc.vector.memset(wz[:], 0.0)
        warm_in = dp.tile([1], F32, name="warm_in")
        warm_out = dp.tile([1], F32, name="warm_out")
        nc.sync.dma_start(warm_in[:], wz[:, 0])
        nc.gpsimd.collective_compute(
            "AllReduce", mybir.AluOpType.max,
            replica_groups=[list(range(N_CORES))],
            ins=[warm_in[:]], outs=[warm_out[:]])

        # ---- Phase A: M_i[2048, 256] = W.T @ cosS, two halves of 8 l-tiles.
        # cos slice resident in SBUF; W streamed per half as a few large
        # per-partition-contiguous DMAs. First-needed pieces (cos ft0 on
        # scalar, W ft0 on sync) issue ahead of the bulk so the first matmul
        # fires early. Each accumulation owns a full PSUM bank (matmul
        # start=True clears the whole bank).
        cbig = cp.tile([128, FT * CS], BF16, name="cosr")
        msb = [mp.tile([128, CS], BF16, name=f"msb{lt}") for lt in range(LT)]
        wbigs = [wp.tile([128, FT * 8 * 128], BF16, tag="w", name=f"wbig{h}")
                 for h in range(2)]
        # h0 feed, interleaved so each f-tile lands just ahead of its matmuls
        nc.scalar.dma_start(cbig[:, 0:CS], cosS[:, 0:CS])
        nc.sync.dma_start(wbigs[0][:, 0:1024], Wb[0, :, 0:1024])
        nc.scalar.dma_start(wbigs[0][:, 1024:2048], Wb[0, :, 1024:2048])
        nc.sync.dma_start(cbig[:, CS:5 * CS], cosS[:, CS:5 * CS])
        nc.sync.dma_start(wbigs[0][:, 2048:4096], Wb[0, :, 2048:4096])
        nc.scalar.dma_start(cbig[:, 5 * CS:FT * CS], cosS[:, 5 * CS:FT * CS])
        nc.scalar.dma_start(wbigs[0][:, 4096:6144], Wb[0, :, 4096:6144])
        nc.sync.dma_start(wbigs[0][:, 6144:9216], Wb[0, :, 6144:9216])
        nc.scalar.dma_start(wbigs[0][:, 9216:13312], Wb[0, :, 9216:13312])
        nc.sync.dma_start(wbigs[0][:, 13312:FT * 1024], Wb[0, :, 13312:FT * 1024])
        WCH = [(0, 3), (3, 7), (7, 11), (11, 14), (14, 17)]
        for h in range(2):
            pa = [ps.tile([128, 512], F32, tag="ps", name=f"pa{h}_{k}")
                  for k in range(8)]
            wbig = wbigs[h]
            if h == 1:
                for n, (f0, f1) in enumerate(WCH):
                    qs[n % 2].dma_start(wbig[:, f0 * 1024:f1 * 1024],
                                        Wb[h, :, f0 * 1024:f1 * 1024])
            for ft in range(FT):
                for lk in range(8):
                    nc.tensor.matmul(
                        pa[lk][:, 0:CS],
                        wbig[:, ft * 1024 + lk * 128:ft * 1024 + (lk + 1) * 128],
                        cbig[:, ft * CS:(ft + 1) * CS],
                        start=(ft == 0), stop=(ft == FT - 1))
            for k in range(8):
                if k % 2 == 0:
                    nc.vector.tensor_copy(msb[h * 8 + k][:], pa[k][:, 0:CS])
                else:
                    nc.scalar.copy(msb[h * 8 + k][:], pa[k][:, 0:CS])

        # ---- Phase B: outT[256, 4096] = M_i.T @ x, two passes of 4 m-chunks
        ot = [op.tile([128, B], F32, name=f"ot{ct}") for ct in range(2)]
        vmaxes = sp.tile([128, 16], F32)
        cco2 = dp.tile([1], F32, name="cco2")
        for p in range(2):
            pb = [ps.tile([128, 512], F32, tag="ps", name=f"pb{p}_{j}")
                  for j in range(8)]
            for lt in range(LT):
                xt = xp.tile([128, 2048], BF16, tag="x")
                qs[lt % 2].dma_start(xt[:], xT[lt, :, p * 2048:(p + 1) * 2048])
                for ct in range(2):
                    lhsT = msb[lt][:, ct * 128:(ct + 1) * 128]
                    for mc in range(4):
                        nc.tensor.matmul(
                            pb[ct * 4 + mc][:], lhsT,
                            xt[:, mc * 512:(mc + 1) * 512],
                            start=(lt == 0), stop=(lt == LT - 1))
            # reduce maxes straight from PSUM (vector); a single final
            # AllReduce covers all 16 -- with the warmup having absorbed the
            # bootstrap, an intermediate stage only lengthens the serialized
            # cc-stream chain
            for j in range(8):
                nc.vector.reduce_max(vmaxes[:, p * 8 + j:p * 8 + j + 1],
                                     pb[j][:], axis=mybir.AxisListType.X)
            if p == 1:
                g2 = sp.tile([1, 1], F32, name="g_s2")
                nc.gpsimd.reduce_max(g2[:], vmaxes[:],
                                     axis=mybir.AxisListType.XYZWC)
                cc_in2 = dp.tile([1], F32, name="ccin2")
                nc.scalar.dma_start(cc_in2[:], g2[:, 0])
                nc.gpsimd.collective_compute(
                    "AllReduce", mybir.AluOpType.max,
                    replica_groups=[list(range(N_CORES))],
                    ins=[cc_in2[:]], outs=[cco2[:]])
            for j in range(8):
                ct, mc = j // 4, j % 4
                k = p * 4 + mc
                osl = ot[ct][:, k * 512:(k + 1) * 512]
                if j % 2 == 0:
                    nc.vector.tensor_copy(osl, pb[j][:])
                else:
                    nc.scalar.copy(osl, pb[j][:])

        # broadcast the global max to all partitions and invert
        gbc = sp.tile([128, 1], F32)
        nc.sync.dma_start(gbc[:], cco2[:].partition_broadcast(128))
        rbc = sp.tile([128, 1], F32)
        nc.vector.reciprocal(rbc[:], gbc[:])

        # scale (f32 -> bf16) + store in [128, 1024] chunks; muls split
        # DVE/Activation (5/3 balances their rates), stores all on sync
        ots = [op.tile([128, B], BF16, name=f"ots{ct}") for ct in range(2)]
        order = [(ct, kk) for ct in range(2) for kk in range(4)]
        for n, (ct, kk) in enumerate(order):
            sl = slice(kk * 1024, (kk + 1) * 1024)
            if n in (1, 4, 6):
                nc.scalar.mul(ots[ct][:, sl], ot[ct][:, sl], rbc[:, 0:1])
            else:
                nc.vector.tensor_scalar_mul(ots[ct][:, sl], ot[ct][:, sl],
                                            rbc[:, 0:1])
            nc.sync.dma_start(
                out[ct * 128:(ct + 1) * 128, sl], ots[ct][:, sl])


def _build():
    nc = bacc.Bacc("TRN2", target_bir_lowering=False, debug=False,
                   num_devices=N_CORES)
    xT = nc.dram_tensor("xT", [LT, 128, B], BF16, kind="ExternalInput")
    # Wb[h, p, ft*8*128 + lk*128 + b] = Wp[ft*128+p, (h*8+lk)*128+b]
    Wb = nc.dram_tensor("Wb", [2, 128, FT * 8 * 128], BF16,
                        kind="ExternalInput")
    # cosS[p, ft*CS + c] = cosp[ft*128+p, core_lo + c]
    cosS = nc.dram_tensor("cosS", [128, FT * CS], BF16, kind="ExternalInput")
    out = nc.dram_tensor("out", [CS, B], BF16, kind="ExternalOutput")
    with tile.TileContext(nc) as tc:
        _emit(nc, tc, xT, Wb, cosS, out)
    nc.compile()
    return nc


_cached_nc = None


def _get_nc():
    global _cached_nc
    if _cached_nc is None:
        _cached_nc = _build()
    return _cached_nc


def _bf16(a: np.ndarray) -> np.ndarray:
    return np.ascontiguousarray(a, dtype=np.float32).astype(ml_dtypes.bfloat16)


def _prep_inputs(x, W, cos_basis):
    x = np.ascontiguousarray(x, dtype=np.float32)
    W = np.ascontiguousarray(W, dtype=np.float32)
    cos = np.ascontiguousarray(cos_basis, dtype=np.float32)
    Wp = np.zeros((FP, L), dtype=np.float32)
    Wp[:F] = W
    cosp = np.zeros((FP, L), dtype=np.float32)
    cosp[:F] = cos
    # Wb[h, p, (ft, lk, b)] = Wp[ft*128+p, (h*8+lk)*128+b]
    W4 = Wp.reshape(FT, 128, LT, 128)
    Wb = _bf16(np.stack([
        np.ascontiguousarray(
            W4[:, :, h * 8:(h + 1) * 8, :].transpose(1, 0, 2, 3)
        ).reshape(128, FT * 8 * 128)
        for h in range(2)]))
    xTf = _bf16(np.ascontiguousarray(x.T).reshape(LT, 128, B))
    # cosS[p, (ft, c)] = cosp[ft*128+p, i*CS+c]
    cosSs = [_bf16(np.ascontiguousarray(
        cosp[:, i * CS:(i + 1) * CS].reshape(FT, 128, CS).transpose(1, 0, 2)
    ).reshape(128, FT * CS)) for i in range(N_CORES)]
    return xTf, Wb, cosSs


def kernel(x, W, cos_basis, _trace=False, _trace_kwargs=None):
    xTf, Wb, cosSs = _prep_inputs(x, W, cos_basis)
    nc = _get_nc()
    in_maps = [{"xT": xTf, "Wb": Wb, "cosS": cosSs[i]}
               for i in range(N_CORES)]
    res = bass_utils.run_bass_kernel_spmd(
        nc, in_maps, core_ids=list(range(N_CORES)), trace=_trace,
        **(_trace_kwargs or {}))
    full = np.empty((B, L), dtype=np.float32)
    for i in range(N_CORES):
        full[:, i * CS:(i + 1) * CS] = res.results[i]["out"].astype(np.float32).T
    if _trace:
        kernel.last_result = res
    return full
